# revision 53
# baseline (speedup 1.0000x reference)
"""BinConv (binarize-both-operands 3x3 conv, stride 1, pad 1) on 8 trn2 cores.

Strategy: data-parallel over batch (4 images per core), weights replicated.

Per-core device kernel:
  - x chunk DMA'd in as fp32, binarized with one exact DVE op
    (is_ge 0.0, subtract 0.5) -> {-0.5, +0.5} in fp8e4.
  - Weights arrive host-transposed as [c_in, tap, c_out] fp32, binarized on
    device to {-2, +2} fp8e4 (so x*w products are exactly +-1; PSUM fp32
    accumulation of <= 1152 such products is exact).
  - The image sits in a fully zero-padded fp8 buffer (114x114 per image:
    zero row above/below, zero column left/right), so each of the 9 taps is
    a strided-AP matmul with no edge corrections at all.
  - PSUM -> SBUF with bias via ACT activation(Identity, bias), then DMA out.
"""

import os
import sys

import numpy as np

for _p in ("/opt/trn_rl_repo", "/opt/pypackages"):
    if _p not in sys.path and os.path.isdir(_p):
        sys.path.append(_p)

from concourse import bacc, bass, mybir, tile  # noqa: E402
from concourse.ap import AP  # noqa: E402
from concourse.bass_utils import run_bass_kernel_spmd  # noqa: E402

F32 = mybir.dt.float32
F8 = mybir.dt.float8e4
ALU = mybir.AluOpType
ACTF = mybir.ActivationFunctionType

N_CORES = 8
P = 128  # C_in == C_out == partitions
H = W = 112
HWIMG = H * W  # 12544
IMGS = 4  # images per core
QROWS = 28  # rows per DMA chunk / output quarter
CHUNK = QROWS * W  # 3136
NTILE = 448  # matmul free dim (4 output rows), one PSUM bank
TROWS = NTILE // W  # 4
TILES_PER_CHUNK = CHUNK // NTILE  # 7
RS = W + 2  # padded row stride (112 data + zero col each side)
TSIZE = (H + 2) * RS  # 114*114 = 12996

# tap t = (kh, kw); for the output tile starting at row r0, tap t reads the
# padded buffer at base (r0+kh)*RS + kw with free dims [TROWS @ RS, W @ 1]
OFF = [(t // 3) * RS + (t % 3) for t in range(9)]

# matmul variant: "A" = 9 single matmuls; "C" = 4 DoubleRow lexicographic
# pairs + 1 single (rhs pair strides 1/112/1/1).
VARIANT = os.environ.get("BINCONV_VARIANT", "C")


def _rhs_ap(T: bass.AP, base: int, pair_d: int | None) -> bass.AP:
    """Strided tap view of the padded image buffer: [P, (2,) TROWS, W]."""
    pstride = list(T.ap[0])
    dims = [pstride]
    if pair_d is not None:
        dims.append([pair_d, 2])
    dims += [[RS, TROWS], [1, W]]
    return AP(T.tensor, base, dims)


def _emit_main_matmuls(nc, ps_list, wb2, T, r0_list, variant):
    """Accumulate all 9 taps into each PSUM tile (one per output row-group).

    Loops weight-sets outermost so consecutive matmuls share the stationary
    operand (amortizes LDWEIGHTS across the tiles in the group).
    """
    dr = mybir.MatmulPerfMode.DoubleRow
    if variant == "A":
        groups = [((t,), False) for t in range(9)]
    elif variant == "C":
        groups = [((2 * p, 2 * p + 1), True) for p in range(4)] + [((8,), False)]
    else:
        raise ValueError(variant)
    for g, (taps, is_pair) in enumerate(groups):
        t = taps[0]
        if is_pair:
            step = taps[1] - taps[0]
            lhsT = wb2[:, t : t + step + 1 : step, :]
        else:
            lhsT = wb2[:, t, :]
        for ps, r0 in zip(ps_list, r0_list):
            kh, kw = t // 3, t % 3
            base = (r0 + kh) * RS + kw
            rhs = _rhs_ap(T, base, (OFF[taps[1]] - OFF[t]) if is_pair else None)
            nc.tensor.matmul(
                ps[:],
                lhsT,
                rhs,
                start=(g == 0),
                stop=(g == len(groups) - 1),
                perf_mode=dr if is_pair else None,
            )


def build(n_imgs=IMGS, variant=VARIANT, n_cores=N_CORES):
    nc = bacc.Bacc(
        "TRN2", target_bir_lowering=False, debug=False, num_devices=n_cores
    )
    x_ext = nc.declare_dram_parameter("x", [n_imgs, P, H, W], F32, isOutput=False)
    wt_ext = nc.declare_dram_parameter("wt", [P, 9, P], F32, isOutput=False)
    b_ext = nc.declare_dram_parameter("b", [P, 1], F32, isOutput=False)
    out_ext = nc.declare_dram_parameter("out", [n_imgs, P, H, W], F32, isOutput=True)

    with tile.TileContext(nc) as tc:
        with (
            tc.tile_pool(name="wpool", bufs=1) as wpool,
            tc.tile_pool(name="inpool", bufs=4) as inpool,
            tc.tile_pool(name="tpool", bufs=4) as tpool,
            tc.tile_pool(name="outpool", bufs=5) as outpool,
            tc.tile_pool(name="pspool", bufs=6, space="PSUM") as pspool,
        ):
            # ---- weights / bias prep (one-time; DMA'd on the scalar ring so
            # the x chunks own the sync ring) ----
            # dependency-free DVE warmup: pays the first-instruction fetch
            # stall at t~0 instead of in front of the first binarize
            warm = wpool.tile([P, 4], F8)
            nc.vector.memset(warm[:], 0.0)
            wt_stage = wpool.tile([P, 9 * P], F32)
            nc.scalar.dma_start(wt_stage[:], wt_ext[:])
            bias = wpool.tile([P, 1], F32)
            nc.scalar.dma_start(bias[:], b_ext[:])
            wb2 = None  # weight prep emitted after image 0's binarize ops

            for img in range(n_imgs):
                # ---- load + binarize into zero-padded buffer ----
                T = tpool.tile([P, TSIZE], F8)
                nc.gpsimd.memset(T[:, 0:RS], 0.0)  # top zero row
                nc.gpsimd.memset(T[:, TSIZE - RS : TSIZE], 0.0)  # bottom zero row
                nc.gpsimd.memset(T[:, 0 : TSIZE - RS + 1 : RS], 0.0)  # left zeros
                nc.gpsimd.memset(T[:, RS - 1 : TSIZE : RS], 0.0)  # right zeros
                # image 0 gets a small leading chunk so the first matmul
                # group unblocks as early as possible
                row_splits = [0, 16, 44, 72, 100, 112] if img == 0 else [
                    0, 28, 56, 84, 112
                ]
                for r_lo, r_hi in zip(row_splits, row_splits[1:]):
                    nrows = r_hi - r_lo
                    xin = inpool.tile([P, QROWS * W], F32, name="xin", tag="xin")
                    # sync engine does nothing else -> input DMA dispatch is
                    # never gated behind compute in an engine FIFO
                    nc.sync.dma_start(
                        xin[:, : nrows * W], x_ext[img, :, r_lo:r_hi, :]
                    )
                    dst = AP(
                        T[:].tensor,
                        (r_lo + 1) * RS + 1,
                        [list(T[:].ap[0]), [RS, nrows], [1, W]],
                    )
                    nc.vector.tensor_scalar(
                        dst, xin[:, : nrows * W], 0.0, 0.5, ALU.is_ge, ALU.subtract
                    )

                if wb2 is None:
                    # emitted after image 0's binarize ops so DVE starts on
                    # the input stream the moment chunk 0 lands
                    whalf = wpool.tile([P, 9 * P], F8)  # {-0.5, +0.5}
                    nc.vector.tensor_scalar(
                        whalf[:], wt_stage[:], 0.0, 0.5, ALU.is_ge, ALU.subtract
                    )
                    wb2 = wpool.tile([P, 9, P], F8)  # {-2, +2}
                    nc.vector.tensor_scalar_mul(
                        wb2[:], whalf[:].rearrange("p (t c) -> p t c", t=9), 4.0
                    )

                # ---- main conv tiles (groups of 3 share LDWEIGHTS) ----
                for q in range(4):
                    outsb = outpool.tile([P, CHUNK], F32)
                    for s0 in range(0, TILES_PER_CHUNK, 3):
                        snames = list(range(s0, min(s0 + 3, TILES_PER_CHUNK)))
                        ps_list = [
                            pspool.tile([P, NTILE], F32, name=f"ps{i}", tag="ps")
                            for i in range(len(snames))
                        ]
                        r0_list = [q * QROWS + s * TROWS for s in snames]
                        _emit_main_matmuls(nc, ps_list, wb2, T, r0_list, variant)
                        for ps, s in zip(ps_list, snames):
                            nc.scalar.activation(
                                outsb[:, s * NTILE : (s + 1) * NTILE],
                                ps[:],
                                ACTF.Identity,
                                bias=bias[:],
                            )
                    nc.scalar.dma_start(
                        out_ext[img, :, q * QROWS : (q + 1) * QROWS, :], outsb[:]
                    )

    nc.compile()
    return nc


def _host_prep(x, W_, b):
    x = np.ascontiguousarray(np.asarray(x, dtype=np.float32))
    W_ = np.asarray(W_, dtype=np.float32)
    b = np.asarray(b, dtype=np.float32)
    # [C_out, C_in, 3, 3] -> [C_in, tap, C_out] (pure layout change)
    wt = np.ascontiguousarray(np.transpose(W_, (1, 2, 3, 0)).reshape(P, 9, P))
    b2 = np.ascontiguousarray(b.reshape(P, 1))
    return x, wt, b2


def run(x, W, b, trace=False, variant=VARIANT, trace_cores=None):
    x, wt, b2 = _host_prep(x, W, b)
    n = x.shape[0]
    per = n // N_CORES
    nc = build(n_imgs=per, variant=variant)
    in_maps = [
        {"x": np.ascontiguousarray(x[k * per : (k + 1) * per]), "wt": wt, "b": b2}
        for k in range(N_CORES)
    ]
    kwargs = {"trace_cores": trace_cores} if trace_cores else {}
    res = run_bass_kernel_spmd(nc, in_maps, list(range(N_CORES)), trace=trace, **kwargs)
    out = np.concatenate([res.results[k]["out"] for k in range(N_CORES)], axis=0)
    return out, res


def kernel(x, W, b):
    out, _ = run(x, W, b, trace=False)
    return out


if __name__ == "__main__":
    xs = np.random.randn(32, P, H, W).astype(np.float32)
    Ws = np.random.randn(P, P, 3, 3).astype(np.float32) * 0.03
    bs = np.random.randn(P).astype(np.float32) * 0.01
    out = kernel(xs, Ws, bs)
    print(out.shape, out.dtype)


# revision 54
# speedup vs baseline: 1.0053x; 1.0053x over previous
"""BinConv (binarize-both-operands 3x3 conv, stride 1, pad 1) on 8 trn2 cores.

Strategy: data-parallel over batch (4 images per core), weights replicated.

Per-core device kernel:
  - x chunk DMA'd in as fp32, binarized with one exact DVE op
    (is_ge 0.0, subtract 0.5) -> {-0.5, +0.5} in fp8e4.
  - Weights arrive host-transposed as [c_in, tap, c_out] fp32, binarized on
    device to {-2, +2} fp8e4 (so x*w products are exactly +-1; PSUM fp32
    accumulation of <= 1152 such products is exact).
  - The image sits in a fully zero-padded fp8 buffer (114x114 per image:
    zero row above/below, zero column left/right), so each of the 9 taps is
    a strided-AP matmul with no edge corrections at all.
  - PSUM -> SBUF with bias via ACT activation(Identity, bias), then DMA out.
"""

import os
import sys

import numpy as np

for _p in ("/opt/trn_rl_repo", "/opt/pypackages"):
    if _p not in sys.path and os.path.isdir(_p):
        sys.path.append(_p)

from concourse import bacc, bass, mybir, tile  # noqa: E402
from concourse.ap import AP  # noqa: E402
from concourse.bass_utils import run_bass_kernel_spmd  # noqa: E402

F32 = mybir.dt.float32
F8 = mybir.dt.float8e4
ALU = mybir.AluOpType
ACTF = mybir.ActivationFunctionType

N_CORES = 8
P = 128  # C_in == C_out == partitions
H = W = 112
HWIMG = H * W  # 12544
IMGS = 4  # images per core
QROWS = 28  # rows per DMA chunk / output quarter
CHUNK = QROWS * W  # 3136
NTILE = 448  # matmul free dim (4 output rows), one PSUM bank
TROWS = NTILE // W  # 4
TILES_PER_CHUNK = CHUNK // NTILE  # 7
RS = W + 2  # padded row stride (112 data + zero col each side)
TSIZE = (H + 2) * RS  # 114*114 = 12996

# tap t = (kh, kw); for the output tile starting at row r0, tap t reads the
# padded buffer at base (r0+kh)*RS + kw with free dims [TROWS @ RS, W @ 1]
OFF = [(t // 3) * RS + (t % 3) for t in range(9)]

# matmul variant: "A" = 9 single matmuls; "C" = 4 DoubleRow lexicographic
# pairs + 1 single (rhs pair strides 1/112/1/1).
VARIANT = os.environ.get("BINCONV_VARIANT", "C")


def _rhs_ap(T: bass.AP, base: int, pair_d: int | None) -> bass.AP:
    """Strided tap view of the padded image buffer: [P, (2,) TROWS, W]."""
    pstride = list(T.ap[0])
    dims = [pstride]
    if pair_d is not None:
        dims.append([pair_d, 2])
    dims += [[RS, TROWS], [1, W]]
    return AP(T.tensor, base, dims)


def _emit_main_matmuls(nc, ps_list, wb2, T, r0_list, variant):
    """Accumulate all 9 taps into each PSUM tile (one per output row-group).

    Loops weight-sets outermost so consecutive matmuls share the stationary
    operand (amortizes LDWEIGHTS across the tiles in the group).
    """
    dr = mybir.MatmulPerfMode.DoubleRow
    if variant == "A":
        groups = [((t,), False) for t in range(9)]
    elif variant == "C":
        groups = [((2 * p, 2 * p + 1), True) for p in range(4)] + [((8,), False)]
    else:
        raise ValueError(variant)
    for g, (taps, is_pair) in enumerate(groups):
        t = taps[0]
        if is_pair:
            step = taps[1] - taps[0]
            lhsT = wb2[:, t : t + step + 1 : step, :]
        else:
            lhsT = wb2[:, t, :]
        for ps, r0 in zip(ps_list, r0_list):
            kh, kw = t // 3, t % 3
            base = (r0 + kh) * RS + kw
            rhs = _rhs_ap(T, base, (OFF[taps[1]] - OFF[t]) if is_pair else None)
            nc.tensor.matmul(
                ps[:],
                lhsT,
                rhs,
                start=(g == 0),
                stop=(g == len(groups) - 1),
                perf_mode=dr if is_pair else None,
            )


def build(n_imgs=IMGS, variant=VARIANT, n_cores=N_CORES):
    nc = bacc.Bacc(
        "TRN2", target_bir_lowering=False, debug=False, num_devices=n_cores
    )
    x_ext = nc.declare_dram_parameter("x", [n_imgs, P, H, W], F32, isOutput=False)
    wt_ext = nc.declare_dram_parameter("wt", [P, 9, P], F32, isOutput=False)
    b_ext = nc.declare_dram_parameter("b", [P, 1], F32, isOutput=False)
    out_ext = nc.declare_dram_parameter("out", [n_imgs, P, H, W], F32, isOutput=True)

    with tile.TileContext(nc) as tc:
        with (
            tc.tile_pool(name="wpool", bufs=1) as wpool,
            tc.tile_pool(name="inpool", bufs=4) as inpool,
            tc.tile_pool(name="tpool", bufs=4) as tpool,
            tc.tile_pool(name="outpool", bufs=5) as outpool,
            tc.tile_pool(name="pspool", bufs=7, space="PSUM") as pspool,
        ):
            # ---- weights / bias prep (one-time; DMA'd on the scalar ring so
            # the x chunks own the sync ring) ----
            # dependency-free DVE warmup: pays the first-instruction fetch
            # stall at t~0 instead of in front of the first binarize
            warm = wpool.tile([P, 4], F8)
            nc.vector.memset(warm[:], 0.0)
            wt_stage = wpool.tile([P, 9 * P], F32)
            nc.scalar.dma_start(wt_stage[:], wt_ext[:])
            bias = wpool.tile([P, 1], F32)
            nc.scalar.dma_start(bias[:], b_ext[:])
            wb2 = None  # weight prep emitted after image 0's binarize ops

            for img in range(n_imgs):
                # ---- load + binarize into zero-padded buffer ----
                T = tpool.tile([P, TSIZE], F8)
                nc.gpsimd.memset(T[:, 0:RS], 0.0)  # top zero row
                nc.gpsimd.memset(T[:, TSIZE - RS : TSIZE], 0.0)  # bottom zero row
                nc.gpsimd.memset(T[:, 0 : TSIZE - RS + 1 : RS], 0.0)  # left zeros
                nc.gpsimd.memset(T[:, RS - 1 : TSIZE : RS], 0.0)  # right zeros
                # image 0 gets a small leading chunk so the first matmul
                # group unblocks as early as possible
                row_splits = [0, 16, 44, 72, 100, 112] if img == 0 else [
                    0, 28, 56, 84, 112
                ]
                for r_lo, r_hi in zip(row_splits, row_splits[1:]):
                    nrows = r_hi - r_lo
                    xin = inpool.tile([P, QROWS * W], F32, name="xin", tag="xin")
                    # sync engine does nothing else -> input DMA dispatch is
                    # never gated behind compute in an engine FIFO
                    nc.sync.dma_start(
                        xin[:, : nrows * W], x_ext[img, :, r_lo:r_hi, :]
                    )
                    dst = AP(
                        T[:].tensor,
                        (r_lo + 1) * RS + 1,
                        [list(T[:].ap[0]), [RS, nrows], [1, W]],
                    )
                    nc.vector.tensor_scalar(
                        dst, xin[:, : nrows * W], 0.0, 0.5, ALU.is_ge, ALU.subtract
                    )

                if wb2 is None:
                    # emitted after image 0's binarize ops so DVE starts on
                    # the input stream the moment chunk 0 lands
                    whalf = wpool.tile([P, 9 * P], F8)  # {-0.5, +0.5}
                    nc.vector.tensor_scalar(
                        whalf[:], wt_stage[:], 0.0, 0.5, ALU.is_ge, ALU.subtract
                    )
                    wb2 = wpool.tile([P, 9, P], F8)  # {-2, +2}
                    nc.vector.tensor_scalar_mul(
                        wb2[:], whalf[:].rearrange("p (t c) -> p t c", t=9), 4.0
                    )

                # ---- main conv tiles (groups of 3 share LDWEIGHTS) ----
                for q in range(4):
                    outsb = outpool.tile([P, CHUNK], F32)
                    for s0 in range(0, TILES_PER_CHUNK, 3):
                        snames = list(range(s0, min(s0 + 3, TILES_PER_CHUNK)))
                        ps_list = [
                            pspool.tile([P, NTILE], F32, name=f"ps{i}", tag="ps")
                            for i in range(len(snames))
                        ]
                        r0_list = [q * QROWS + s * TROWS for s in snames]
                        _emit_main_matmuls(nc, ps_list, wb2, T, r0_list, variant)
                        for ps, s in zip(ps_list, snames):
                            nc.scalar.activation(
                                outsb[:, s * NTILE : (s + 1) * NTILE],
                                ps[:],
                                ACTF.Identity,
                                bias=bias[:],
                            )
                    nc.scalar.dma_start(
                        out_ext[img, :, q * QROWS : (q + 1) * QROWS, :], outsb[:]
                    )

    nc.compile()
    return nc


def _host_prep(x, W_, b):
    x = np.ascontiguousarray(np.asarray(x, dtype=np.float32))
    W_ = np.asarray(W_, dtype=np.float32)
    b = np.asarray(b, dtype=np.float32)
    # [C_out, C_in, 3, 3] -> [C_in, tap, C_out] (pure layout change)
    wt = np.ascontiguousarray(np.transpose(W_, (1, 2, 3, 0)).reshape(P, 9, P))
    b2 = np.ascontiguousarray(b.reshape(P, 1))
    return x, wt, b2


def run(x, W, b, trace=False, variant=VARIANT, trace_cores=None):
    x, wt, b2 = _host_prep(x, W, b)
    n = x.shape[0]
    per = n // N_CORES
    nc = build(n_imgs=per, variant=variant)
    in_maps = [
        {"x": np.ascontiguousarray(x[k * per : (k + 1) * per]), "wt": wt, "b": b2}
        for k in range(N_CORES)
    ]
    kwargs = {"trace_cores": trace_cores} if trace_cores else {}
    res = run_bass_kernel_spmd(nc, in_maps, list(range(N_CORES)), trace=trace, **kwargs)
    out = np.concatenate([res.results[k]["out"] for k in range(N_CORES)], axis=0)
    return out, res


def kernel(x, W, b):
    out, _ = run(x, W, b, trace=False)
    return out


if __name__ == "__main__":
    xs = np.random.randn(32, P, H, W).astype(np.float32)
    Ws = np.random.randn(P, P, 3, 3).astype(np.float32) * 0.03
    bs = np.random.randn(P).astype(np.float32) * 0.01
    out = kernel(xs, Ws, bs)
    print(out.shape, out.dtype)
